# revision 34
# baseline (speedup 1.0000x reference)
"""MiniBatchDiscrimination kernel for 8 Trainium2 NeuronCores.

Reference computation (N=256 samples, A=2048 in_features, B=64 out_features,
C=32 kernel dim):
    M  = (f @ T).reshape(N, B, C)
    L1[i,j,b] = sum_c |M[j,b,c] - M[i,b,c]|
    o[j,b]    = sum_i exp(-L1[i,j,b])        (includes the i==j self term = 1)
    out = concat([f, o], axis=1)

Strategy (retrieval-knn pruning):
  exp(-L1) underflows fp32 to 0 whenever L1 > ~104, and ||v||_1 >= ||v||_2,
  so any pair with squared-L2 distance D2 >= T (T = 110^2) contributes
  exactly 0 to the fp32 sum (< min fp32 denormal).  D2 is computable on the
  TensorEngine at full speed via the Gram matrix:
      D2[i,j,b] = n[i,b] + n[j,b] - 2*G[i,j,b],   G = M_b @ M_b^T
  For N(0,1) random inputs, D2 concentrates around 131k +- 33k, so the only
  pairs with D2 < T are exact duplicates (D2 == 0, for which exp(-L1) == 1
  exactly).  The device kernel therefore computes, for every (j, b):
      o[j,b] = #{ i : D2[i,j,b] < T }
  which equals the reference fp32 result whenever no pair falls in the
  ambiguous band 0 < L1 < ~110.  The host verifies this condition
  (o != 1 anywhere => some near-pair exists) and falls back to an exact
  host-side recomputation of the affected feature columns -- so the result
  is correct for ALL inputs; the fast path is exact for inputs without
  near-duplicate rows (up to a < 1e-40 relative error from dropped
  denormal-sized terms).

Sharding: tensor-parallel over the B*C (=2048) columns of T.  Core d gets
T[:, 256*d : 256*(d+1)] (8 of the 64 b-features), computes M^T for its
block via PE (K=2048 GEMM), then Gram + screen for its 8 b's entirely
locally, and outputs o[:, 8d:8d+8].  No collectives; host concatenates.

Per-core device pipeline (all engines via TileContext auto-sync):
  1. GEMM:  MT = (f @ Tblk)^T  as  out[bc, i] = sum_a Tblk[a, bc] * fT[a, i]
     (lhsT = Tblk tiles, rhs = fT tiles, both bf16, fp32 PSUM accumulate)
  2. sq = MT^2 (fp32), norms both orientations via matmuls with a
     block-indicator constant S:  n_rows (4,256) = S^T @ sq  and
     n_cols (128,4) = sq^T @ S
  3. Per b, per 128-row i-block:  PSUM = Gram (K=32, row-packed via
     tile_position) + (-n_j/2) rank-1 fold (K=1 ones x norm row)
  4. indicator + count:  is_gt against per-partition threshold
     (n_i - T)/2 with free-dim accum (D2 symmetric => row count == col
     count); split across DVE (is_gt) and ACT (Sign + fixup)
  5. DMA o (256, 8) fp32 out.
"""

import os

import ml_dtypes
import numpy as np

N = 256  # batch
A = 2048  # in_features
B = 64  # out_features
C = 32  # kernel dim
NCORES = 8
BLOCAL = B // NCORES  # 8 b-features per core
BCL = BLOCAL * C  # 256 M^T rows per core
KT = A // 128  # 16 k-tiles
# Squared-L2 screen threshold.  Pairs with computed D2 >= T are dropped;
# the bf16 pipeline's worst-case D2 error is ~1.4e3 (measured), so dropped
# pairs have true L1 >= L2 >= sqrt(2500-1400) ~ 33 => contribution < 3e-15.
# Kept far below the observed minimum off-diagonal D2 (~1.65e4) so bf16
# noise can never produce a false survivor (which would only cost a host
# fallback, not correctness).
T_SCREEN = 2500.0

_BF16 = ml_dtypes.bfloat16

_compiled = None
last_run_info = None  # BassKernelResults of the most recent device run


def _emit_body(nc, mybir, inp, work, scr, pbig, pn, consts, fT_d, Tb_d, o_d):
    f32 = mybir.dt.float32
    bf16 = mybir.dt.bfloat16
    S_sb, S128_sb, ones_sb = consts

    # ---- load inputs, chunked so the GEMM starts after the first chunk ----
    # (row a*128+p of DRAM -> tile [p, a, :]); alternate fT/Tb chunks so both
    # operands of k-tile 0 arrive first
    NCH = 4
    KPC = KT // NCH  # k-tiles per chunk
    fT_ch, Tb_ch = [], []
    fT_q = [nc.sync] * NCH
    Tb_q = [nc.gpsimd, nc.scalar, nc.gpsimd, nc.scalar]
    for c in range(NCH):
        ftt = inp.tile([128, KPC, N], bf16, tag=f"fT{c}")
        fT_q[c].dma_start(
            ftt[:], fT_d[:, KPC * N * c : KPC * N * (c + 1)]
        )
        fT_ch.append(ftt)
        tbt = inp.tile([128, KPC, BCL], bf16, tag=f"Tb{c}")
        Tb_q[c].dma_start(
            tbt[:], Tb_d[:, KPC * BCL * c : KPC * BCL * (c + 1)]
        )
        Tb_ch.append(tbt)

    # o packed (128, 16): column 8*mt + b  (single out-DMA)
    o_sb = work.tile([128, 2 * BLOCAL], f32, tag="o")
    tsc_sb = [work.tile([128, 8], f32, tag=f"tsc{mt}", name=f"tsc{mt}") for mt in range(2)]
    tscn_sb = [work.tile([128, 8], f32, tag=f"tscn{mt}", name=f"tscn{mt}") for mt in range(2)]

    # ---- both GEMMs first (PE stays dense; tails overlap on DVE/ACT) ----
    msb_l, ssb_l = [], []
    for t in range(2):
        mtp = pbig.tile([128, N], f32, tag="big")
        for kt in range(KT):
            nc.tensor.matmul(
                mtp[:],
                Tb_ch[kt // KPC][:, kt % KPC, 128 * t : 128 * (t + 1)],
                fT_ch[kt // KPC][:, kt % KPC, :],
                start=(kt == 0),
                stop=(kt == KT - 1),
            )
        msb = scr.tile([128, N], bf16, tag=f"mt{t}", name=f"msb{t}")
        nc.vector.tensor_copy(msb[:], mtp[:])
        # squares of the bf16 M copy on the otherwise-idle GPSIMD engine
        ssb = scr.tile([128, N], bf16, tag=f"sq{t}", name=f"ssb{t}")
        nc.gpsimd.tensor_tensor(ssb[:], msb[:], msb[:], mybir.AluOpType.mult)
        msb_l.append(msb)
        ssb_l.append(ssb)

    ncp_l = [pn.tile([128, 8], f32, tag=f"ncol{mt}", name=f"ncp{mt}", bufs=1) for mt in range(2)]
    nrt_l = []
    for t in range(2):
        ssb = ssb_l[t]
        # norm rows: S128 places -n_b/2 at partition 32*(b%4) directly
        # (rank-1 fold operands must sit in the matmul's row quadrant)
        npp = pn.tile([128, N], f32, tag="nr", bufs=1)
        nc.tensor.matmul(npp[:], S128_sb[:], ssb[:], start=True, stop=True)
        nrt = work.tile([128, N], bf16, tag=f"nrow{t}", name=f"nrt{t}")
        nc.scalar.mul(nrt[:], npp[:], -0.5)
        nrt_l.append(nrt)

        # per-partition norm cols: both t's land in one psum tile per mt
        for mt in range(2):
            nc.tensor.matmul(
                ncp_l[mt][:, 4 * t : 4 * t + 4],
                ssb[:, 128 * mt : 128 * (mt + 1)],
                S_sb[:],
                start=True,
                stop=True,
            )

    # thresholds: tsc = (n_i - T)/2 ; tscn = -tsc  (one copy pair per mt)
    for mt in range(2):
        nc.scalar.activation(
            tsc_sb[mt][:],
            ncp_l[mt][:],
            mybir.ActivationFunctionType.Copy,
            bias=-T_SCREEN / 2.0,
            scale=0.5,
        )
        nc.scalar.activation(
            tscn_sb[mt][:],
            ncp_l[mt][:],
            mybir.ActivationFunctionType.Copy,
            bias=T_SCREEN / 2.0,
            scale=-0.5,
        )

    # gram groups interleaved across t so the pipeline drains evenly
    for g in range(4):
        for t in range(2):
            b = 4 * t + g
            msb, nrt = msb_l[t], nrt_l[t]
            for mt in range(2):
                gp = pbig.tile([128, N], f32, tag="big")
                nc.tensor.matmul(
                    gp[:],
                    msb[32 * g : 32 * g + 32, 128 * mt : 128 * (mt + 1)],
                    msb[32 * g : 32 * g + 32, :],
                    start=True,
                    stop=False,
                    tile_position=(32 * g, 0),
                )
                nc.tensor.matmul(
                    gp[:],
                    ones_sb[32 * g : 32 * g + 1, :],
                    nrt[32 * g : 32 * g + 1, :],
                    start=False,
                    stop=True,
                    tile_position=(32 * g, 0),
                )
                if b % 2 == 0:
                    # DVE: ind = (G' > tsc_i), count = sum_j ind
                    ind = scr.tile([128, N], bf16, tag="ind")
                    nc.vector.tensor_scalar(
                        ind[:],
                        gp[:],
                        tsc_sb[mt][:, b : b + 1],
                        None,
                        mybir.AluOpType.is_gt,
                        mybir.AluOpType.add,
                        accum_out=o_sb[:, 8 * mt + b : 8 * mt + b + 1],
                    )
                else:
                    # ACT: sign(G' - tsc_i) summed; fixed up below
                    ind = scr.tile([128, N], f32, tag="inda")
                    nc.scalar.activation(
                        ind[:],
                        gp[:],
                        mybir.ActivationFunctionType.Sign,
                        bias=tscn_sb[mt][:, b : b + 1],
                        scale=1.0,
                        accum_out=o_sb[:, 8 * mt + b : 8 * mt + b + 1],
                    )

    # ACT columns (odd b -> odd cols) hold sum(sign); count = (x + N) / 2
    nc.vector.tensor_scalar(
        o_sb[:, 1 : 2 * BLOCAL : 2],
        o_sb[:, 1 : 2 * BLOCAL : 2],
        0.5,
        float(N) * 0.5,
        mybir.AluOpType.mult,
        mybir.AluOpType.add,
    )
    nc.sync.dma_start(o_d[:], o_sb[:])


def _build(chain=False, reps=1):
    import concourse.mybir as mybir
    import concourse.tile as tile
    from concourse import bacc

    f32 = mybir.dt.float32
    bf16 = mybir.dt.bfloat16

    nc = bacc.Bacc(None, target_bir_lowering=False, debug=False)
    # host pre-tiles to partition-major: row p holds [x[kt*128+p, :] for kt]
    fT_d = nc.dram_tensor("fT", [128, KT * N], bf16, kind="ExternalInput")
    Tb_d = nc.dram_tensor("Tb", [128, KT * BCL], bf16, kind="ExternalInput")
    o_d = nc.dram_tensor("o", [128, 2 * BLOCAL], f32, kind="ExternalOutput")
    if chain:
        # benchmark-only: data-dependent passthrough for chaining execs
        ch_i = nc.dram_tensor("chain", [128, 16], f32, kind="ExternalInput")
        ch_o = nc.dram_tensor("chain_out", [128, 16], f32, kind="ExternalOutput")

    with tile.TileContext(nc) as tc:
        with (
            tc.tile_pool(name="inp", bufs=2) as inp,
            tc.tile_pool(name="work", bufs=1) as work,
            tc.tile_pool(name="scr", bufs=4) as scr,
            tc.tile_pool(name="pbig", bufs=5, space="PSUM") as pbig,
            tc.tile_pool(name="pn", bufs=2, space="PSUM") as pn,
        ):
            if chain:
                cht = work.tile([128, 16], f32, tag="chain")
                nc.sync.dma_start(cht[:], ch_i[:])
                nc.sync.dma_start(ch_o[:], cht[:])
            # block-indicator consts: S[p, g] = 1 iff p//32 == g, and the
            # 128-wide variant with column 32g live so norm-matmul output
            # rows land at 32-aligned partitions
            S_sb = work.tile([128, 4], bf16, tag="S")
            nc.vector.memset(S_sb[:], 0.0)
            S128_sb = work.tile([128, 128], bf16, tag="S128")
            nc.vector.memset(S128_sb[:], 0.0)
            for g in range(4):
                nc.vector.memset(S_sb[32 * g : 32 * g + 32, g : g + 1], 1.0)
                nc.vector.memset(
                    S128_sb[32 * g : 32 * g + 32, 32 * g : 32 * g + 1], 1.0
                )
            # ones rows at every 32-aligned partition (stationary for the
            # rank-1 -n_j/2 fold; quadrant must match the norm-row quadrant)
            ones_sb = work.tile([128, 128], bf16, tag="ones")
            nc.vector.memset(ones_sb[:], 1.0)

            for _rep in range(reps):
                _emit_body(
                    nc, mybir, inp, work, scr, pbig, pn,
                    (S_sb, S128_sb, ones_sb), fT_d, Tb_d, o_d,
                )

    nc.compile()
    return nc


def _get_compiled():
    global _compiled
    if _compiled is None:
        _compiled = _build()
    return _compiled


def _host_exact_o_column(f64, T64, b):
    """Exact (float64) o[:, b] for one feature column; used only when the
    device screen detects a potential near-duplicate pair."""
    Mb = f64 @ T64[:, C * b : C * (b + 1)]  # (N, C)
    L1 = np.abs(Mb[None, :, :] - Mb[:, None, :]).sum(axis=2)  # (N, N)
    return np.exp(-L1).sum(axis=0)


def _tile_rows(x):
    """(A, W) row-major -> (128, KT*W) partition-major (row p = k-tiles concat)."""
    w = x.shape[1]
    return np.ascontiguousarray(
        x.reshape(KT, 128, w).transpose(1, 0, 2).reshape(128, KT * w)
    )


def make_in_maps(f, T):
    fT = _tile_rows(f.T.astype(_BF16))
    return [
        {
            "fT": fT,
            "Tb": _tile_rows(
                T[:, BCL * d : BCL * (d + 1)].astype(_BF16)
            ),
        }
        for d in range(NCORES)
    ]


def kernel(f, T):
    from concourse.bass_utils import run_bass_kernel_spmd

    global last_run_info
    f = np.asarray(f)
    T = np.asarray(T)
    assert f.shape == (N, A) and T.shape == (A, B * C), (f.shape, T.shape)

    nc = _get_compiled()
    in_maps = make_in_maps(f, T)
    res = run_bass_kernel_spmd(
        nc,
        in_maps,
        core_ids=list(range(NCORES)),
        trace=bool(int(os.environ.get("KERNEL_TRACE", "0"))),
    )
    last_run_info = res

    o = np.empty((N, B), dtype=np.float32)
    for d in range(NCORES):
        od = res.results[d]["o"].reshape(128, 2, BLOCAL)
        o[:, BLOCAL * d : BLOCAL * (d + 1)] = od.transpose(1, 0, 2).reshape(
            N, BLOCAL
        )

    # Screen verification: counts other than 1.0 mean either true duplicates
    # (count k of an identical group => reference sum is also k: exact) or a
    # near-pair in the ambiguous band.  Distinguishing costs more than an
    # exact host recompute of the affected columns, so just recompute those.
    bad_cols = np.where(np.any(o != 1.0, axis=0))[0]
    if bad_cols.size:
        f64 = f.astype(np.float64)
        T64 = T.astype(np.float64)
        for b in bad_cols:
            o[:, b] = _host_exact_o_column(f64, T64, int(b)).astype(np.float32)

    return np.concatenate([f.astype(np.float32, copy=False), o], axis=1)
